# revision 7
# baseline (speedup 1.0000x reference)
"""Trainium2 Bass kernel for nn_BASE_MAMBA_14018773254552.

Mamba block (d_model=128, d_inner=256, d_state=64, d_conv=4, L=1024, B=4)
+ input proj + classifier head.

Sharding: 8 cores = 4 batches x 2 d_inner-halves (128 channels each).
Each core computes its batch's full front-end (input proj, in_proj, conv,
x_proj) feature-major ([feature, time] tiles), then the selective scan for
its 128-channel half, and the partial out-proj + mean-pool. The host sums
the two channel-half partials per batch and runs the tiny classifier
(BatchNorm couples batches, so it cannot live on one core).

Self-contained: hardcodes all shapes; builds + compiles the Bass program
once per process and runs it on cores 0-7 via run_bass_kernel_spmd.
"""
import numpy as np

try:
    import concourse.bacc as bacc
except ImportError:  # pragma: no cover - path fallback
    import sys
    for _p in ("/opt/trn_rl_repo", "/root/.axon_site/_ro/trn_rl_repo"):
        if _p not in sys.path:
            sys.path.insert(0, _p)
    import concourse.bacc as bacc

import ml_dtypes
import concourse.mybir as mybir
import concourse.tile as tile
from concourse.bass_utils import run_bass_kernel_spmd

F32 = mybir.dt.float32
BF16 = mybir.dt.bfloat16
AF = mybir.ActivationFunctionType
OP = mybir.AluOpType

B, L, CIN = 4, 1024, 20
DM, DS, DC = 128, 64, 4
DI = 256
DTR = 8
DH = 128          # channels per core (d_inner half)
EPS = 1e-5

_cache = {}


def _build():
    nc = bacc.Bacc("TRN2", target_bir_lowering=False, debug=False, num_devices=8)

    # ---- I/O ----
    xt_d = nc.dram_tensor("xt", [CIN, L], F32, kind="ExternalInput")
    wpT_d = nc.dram_tensor("wpT", [CIN, DM], F32, kind="ExternalInput")
    bp_d = nc.dram_tensor("bp", [DM, 1], F32, kind="ExternalInput")
    wiT_d = nc.dram_tensor("wiT", [DM, 3 * DH], F32, kind="ExternalInput")
    convw_d = nc.dram_tensor("convw", [DH, 2 * DC], F32, kind="ExternalInput")
    convb_d = nc.dram_tensor("convb", [DH, 2], F32, kind="ExternalInput")
    wxT_d = nc.dram_tensor("wxT", [DH, 2 * 136], F32, kind="ExternalInput")
    wdtT_d = nc.dram_tensor("wdtT", [DTR, DH], F32, kind="ExternalInput")
    bdt_d = nc.dram_tensor("bdt", [DH, 1], F32, kind="ExternalInput")
    alog_d = nc.dram_tensor("alog", [DH, DS], F32, kind="ExternalInput")
    dskip_d = nc.dram_tensor("dskip", [DH, 1], F32, kind="ExternalInput")
    woutT_d = nc.dram_tensor("woutT", [DH, DM], F32, kind="ExternalInput")
    selq_d = nc.dram_tensor("selq", [DS, DS * DH], BF16, kind="ExternalInput")
    pooled_d = nc.dram_tensor("pooled", [DM, 1], F32, kind="ExternalOutput")

    with tile.TileContext(nc) as tc:
        with (
            tc.tile_pool(name="const", bufs=1) as cp,
            tc.tile_pool(name="work", bufs=1) as wp,
        ):
            # ---- load params ----
            xt = cp.tile([CIN, L], F32)
            wpT = cp.tile([CIN, DM], F32)
            bp = cp.tile([DM, 1], F32)
            wiT = cp.tile([DM, 3 * DH], F32)
            convw = cp.tile([DH, 2 * DC], F32)
            convb = cp.tile([DH, 2], F32)
            wxT = cp.tile([DH, 2 * 136], F32)
            wdtT = cp.tile([DTR, DH], F32)
            bdt = cp.tile([DH, 1], F32)
            alog = cp.tile([DH, DS], F32)
            dskip = cp.tile([DH, 1], F32)
            woutT = cp.tile([DH, DM], F32)
            selq = cp.tile([DS, DS * DH], BF16)
            for t_, d_ in [(xt, xt_d), (wpT, wpT_d), (bp, bp_d), (wiT, wiT_d),
                           (convw, convw_d), (convb, convb_d), (wxT, wxT_d),
                           (wdtT, wdtT_d), (bdt, bdt_d), (alog, alog_d),
                           (dskip, dskip_d), (woutT, woutT_d), (selq, selq_d)]:
                nc.sync.dma_start(t_[:], d_[:])

            HLF = (slice(0, 512), slice(512, 1024))

            # ---- phase 1: front-end ----
            with tc.tile_pool(name="ps1", bufs=4, space="PSUM") as ps1:
                # h = Wp @ x + bp   [128 dm, 1024 t]
                h_ps = ps1.tile([DM, L], F32, tag="ps")
                for sl in HLF:
                    nc.tensor.matmul(h_ps[:, sl], wpT[:, :], xt[:, sl])
                h16 = wp.tile([DM, L], F32)
                nc.scalar.activation(h16[:], h_ps[:], AF.Identity, bias=bp[:])

                # xm_j = W_in[chunk_j] @ h   (j=0 own, j=1 other)
                xmp = []   # padded copies in SBUF
                for j in range(2):
                    xm_ps = ps1.tile([DH, L], F32, tag="ps")
                    for sl in HLF:
                        nc.tensor.matmul(
                            xm_ps[:, sl], wiT[:, j * DH:(j + 1) * DH], h16[:, sl])
                    pad = wp.tile([DH, DC - 1 + L], F32, tag=f"xmp{j}")
                    nc.vector.memset(pad[:, 0:DC - 1], 0.0)
                    nc.scalar.copy(pad[:, DC - 1:DC - 1 + L], xm_ps[:])
                    xmp.append(pad)

                # causal depthwise conv + silu -> xc16_j
                xc16 = []
                for j in range(2):
                    cacc = wp.tile([DH, L], F32, tag=f"cacc{j}")
                    nc.vector.tensor_scalar(
                        out=cacc[:], in0=xmp[j][:, 0:L],
                        scalar1=convw[:, 4 * j:4 * j + 1], scalar2=None,
                        op0=OP.mult)
                    for k in (1, 2, 3):
                        nc.vector.scalar_tensor_tensor(
                            out=cacc[:], in0=xmp[j][:, k:k + L],
                            scalar=convw[:, 4 * j + k:4 * j + k + 1],
                            in1=cacc[:], op0=OP.mult, op1=OP.add)
                    xc = wp.tile([DH, L], F32, tag=f"xc{j}")
                    nc.scalar.activation(xc[:], cacc[:], AF.Silu,
                                         bias=convb[:, j:j + 1])
                    xc16.append(xc)

                # dbc = W_x @ xc  -> dtr [8,L], BmT [64,L], CmT [64,L]
                dtr_ps = ps1.tile([DTR, L], F32, tag="ps")
                bm_ps = ps1.tile([DS, L], F32, tag="ps")
                cm_ps = ps1.tile([DS, L], F32, tag="ps")
                for (m0, msz, out_ps) in ((0, DTR, dtr_ps), (DTR, DS, bm_ps),
                                          (DTR + DS, DS, cm_ps)):
                    for sl in HLF:
                        for j in range(2):
                            nc.tensor.matmul(
                                out_ps[:, sl],
                                wxT[:, 136 * j + m0:136 * j + m0 + msz],
                                xc16[j][:, sl],
                                start=(j == 0), stop=(j == 1))
                dtrT = wp.tile([DTR, L], F32)
                nc.scalar.copy(dtrT[:], dtr_ps[:])
                bmT16 = wp.tile([DS, L], BF16)
                nc.scalar.copy(bmT16[:], bm_ps[:])
                cmT16 = wp.tile([DS, L], BF16)
                nc.scalar.copy(cmT16[:], cm_ps[:])

                # dt = softplus(W_dt @ dtr + b_dt); no Softplus ACT table on
                # TRN2, so compute dt_neg = -dt = ln(sigmoid(-(raw + b_dt)))
                # and pair it with +exp(A_log) in the scan exponentials.
                dt_ps = ps1.tile([DH, L], F32, tag="ps")
                for sl in HLF:
                    nc.tensor.matmul(dt_ps[:, sl], wdtT[:, :], dtrT[:, sl])
                bdtn = wp.tile([DH, 1], F32)
                nc.scalar.mul(bdtn[:], bdt[:], -1.0)
                sg = wp.tile([DH, L], F32)
                nc.scalar.activation(sg[:], dt_ps[:], AF.Sigmoid,
                                     bias=bdtn[:], scale=-1.0)
                DT = wp.tile([DH, L], F32)   # holds -dt
                nc.scalar.activation(DT[:], sg[:], AF.Ln)

            # U = dt * xc_own ;  Apos = exp(A_log) ;  Y init = xc_own * Dskip
            U = wp.tile([DH, L], F32)
            nc.vector.scalar_tensor_tensor(
                out=U[:], in0=DT[:], scalar=-1.0, in1=xc16[0][:],
                op0=OP.mult, op1=OP.mult)
            Aneg = wp.tile([DH, DS], F32)    # holds +exp(A_log), pairs with -dt
            nc.scalar.activation(Aneg[:], alog[:], AF.Exp)
            Y = wp.tile([DH, L], F32)
            nc.vector.tensor_scalar(out=Y[:], in0=xc16[0][:], scalar1=dskip[:],
                                    scalar2=None, op0=OP.mult)

            # ---- phase 2: selective scan over 64 state dims ----
            with (
                tc.tile_pool(name="psl", bufs=2, space="PSUM") as psl,
                tc.tile_pool(name="sl", bufs=3) as slp,
            ):
                for n in range(DS):
                    bmb = psl.tile([DH, L], F32, tag="bmb")
                    ccb = psl.tile([DH, L], F32, tag="ccb")
                    seln = selq[:, DH * n:DH * (n + 1)]
                    for sl in HLF:
                        nc.tensor.matmul(bmb[:, sl], seln, bmT16[:, sl])
                        nc.tensor.matmul(ccb[:, sl], seln, cmT16[:, sl])
                    dAt = slp.tile([DH, L], F32, tag="dA")
                    nc.scalar.activation(dAt[:], DT[:], AF.Exp,
                                         scale=Aneg[:, n:n + 1])
                    dBxt = slp.tile([DH, L], F32, tag="dBx")
                    nc.vector.tensor_tensor(out=dBxt[:], in0=U[:], in1=bmb[:],
                                            op=OP.mult)
                    Ht = slp.tile([DH, L], F32, tag="H")
                    nc.vector.tensor_tensor_scan(
                        out=Ht[:], data0=dAt[:], data1=dBxt[:], initial=0.0,
                        op0=OP.mult, op1=OP.add)
                    HCt = slp.tile([DH, L], F32, tag="HC")
                    nc.vector.tensor_tensor(out=HCt[:], in0=Ht[:], in1=ccb[:],
                                            op=OP.mult)
                    nc.gpsimd.tensor_tensor(out=Y[:], in0=Y[:], in1=HCt[:],
                                            op=OP.add)

            # ---- tail: gate, out-proj, pool ----
            with tc.tile_pool(name="ps2", bufs=2, space="PSUM") as ps2:
                z_ps = ps2.tile([DH, L], F32, tag="z")
                for sl in HLF:
                    nc.tensor.matmul(z_ps[:, sl], wiT[:, 2 * DH:3 * DH],
                                     h16[:, sl])
                zsig = wp.tile([DH, L], F32)
                nc.scalar.activation(zsig[:], z_ps[:], AF.Silu)
                y3 = wp.tile([DH, L], F32)
                nc.vector.tensor_tensor(out=y3[:], in0=Y[:], in1=zsig[:],
                                        op=OP.mult)
                out_ps = ps2.tile([DM, L], F32, tag="o")
                for sl in HLF:
                    nc.tensor.matmul(out_ps[:, sl], woutT[:, :], y3[:, sl])
                trash = wp.tile([DM, L], F32)
                pooled = wp.tile([DM, 1], F32)
                nc.scalar.activation(trash[:], out_ps[:], AF.Identity,
                                     scale=1.0 / L, accum_out=pooled[:])
                nc.sync.dma_start(pooled_d[:], pooled[:])

    nc.compile()
    return nc


def _core_inputs(inputs, b, half):
    f32 = np.float32
    bf16 = ml_dtypes.bfloat16
    x = np.asarray(inputs["x"], f32)
    Wp = np.asarray(inputs["Wp"], f32)
    bp = np.asarray(inputs["bp"], f32)
    W_in = np.asarray(inputs["W_in"], f32)
    conv_w = np.asarray(inputs["conv_w"], f32)
    conv_b = np.asarray(inputs["conv_b"], f32)
    W_x = np.asarray(inputs["W_x"], f32)
    W_dt = np.asarray(inputs["W_dt"], f32)
    b_dt = np.asarray(inputs["b_dt"], f32)
    A_log = np.asarray(inputs["A_log"], f32)
    Dskip = np.asarray(inputs["Dskip"], f32)
    W_out = np.asarray(inputs["W_out"], f32)

    own = slice(half * DH, half * DH + DH)
    other = slice(DH, 2 * DH) if half == 0 else slice(0, DH)
    return {
        "xt": np.ascontiguousarray(x[b]),
        "wpT": np.ascontiguousarray(Wp.T),
        "bp": np.ascontiguousarray(bp[:, None]),
        "wiT": np.concatenate(
            [W_in[0:DI][own].T, W_in[0:DI][other].T,
             W_in[DI:2 * DI][own].T], axis=1),
        "convw": np.concatenate([conv_w[own], conv_w[other]], axis=1),
        "convb": np.stack([conv_b[own], conv_b[other]], axis=1),
        "wxT": np.concatenate([W_x.T[own], W_x.T[other]], axis=1),
        "wdtT": np.ascontiguousarray(W_dt[own].T),
        "bdt": np.ascontiguousarray(b_dt[own][:, None]),
        "alog": np.ascontiguousarray(A_log[own]),
        "dskip": np.ascontiguousarray(Dskip[own][:, None]),
        "woutT": np.ascontiguousarray(W_out[:, own].T),
        "selq": np.repeat(np.eye(DS, dtype=bf16), DH, axis=1),
    }


def kernel(**inputs) -> np.ndarray:
    if "nc" not in _cache:
        _cache["nc"] = _build()
    nc = _cache["nc"]

    in_maps = [_core_inputs(inputs, c // 2, c % 2) for c in range(8)]
    res = run_bass_kernel_spmd(nc, in_maps, core_ids=list(range(8)))

    pooled = np.zeros((B, DM), np.float32)
    for c in range(8):
        pooled[c // 2] += res.results[c]["pooled"][:, 0]

    # classifier head (host: BatchNorm couples all batches; ~300 flops)
    f32 = np.float32
    W1 = np.asarray(inputs["W1"], f32)
    b1 = np.asarray(inputs["b1"], f32)
    gamma = np.asarray(inputs["gamma"], f32)
    beta = np.asarray(inputs["beta"], f32)
    W2 = np.asarray(inputs["W2"], f32)
    b2 = np.asarray(inputs["b2"], f32)
    h1 = pooled @ W1.T + b1
    mu = h1.mean(axis=0)
    var = h1.var(axis=0)
    h1 = (h1 - mu) / np.sqrt(var + EPS) * gamma + beta
    h1 = np.maximum(h1, 0.0)
    return (h1 @ W2.T + b2).astype(np.float32)


# revision 8
# speedup vs baseline: 1.0508x; 1.0508x over previous
"""Trainium2 Bass kernel for nn_BASE_MAMBA_14018773254552.

Mamba block (d_model=128, d_inner=256, d_state=64, d_conv=4, L=1024, B=4)
+ input proj + classifier head.

Sharding: 8 cores = 4 batches x 2 d_inner-halves (128 channels each).
Each core computes its batch's full front-end (input proj, in_proj, conv,
x_proj) feature-major ([feature, time] tiles), then the selective scan for
its 128-channel half, and the partial out-proj + mean-pool. The host sums
the two channel-half partials per batch and runs the tiny classifier
(BatchNorm couples batches, so it cannot live on one core).

Self-contained: hardcodes all shapes; builds + compiles the Bass program
once per process and runs it on cores 0-7 via run_bass_kernel_spmd.
"""
import numpy as np

try:
    import concourse.bacc as bacc
except ImportError:  # pragma: no cover - path fallback
    import sys
    for _p in ("/opt/trn_rl_repo", "/root/.axon_site/_ro/trn_rl_repo"):
        if _p not in sys.path:
            sys.path.insert(0, _p)
    import concourse.bacc as bacc

import ml_dtypes
import concourse.bass as bass
import concourse.mybir as mybir
import concourse.tile as tile
from concourse.bass_utils import run_bass_kernel_spmd

F32 = mybir.dt.float32
BF16 = mybir.dt.bfloat16
AF = mybir.ActivationFunctionType
OP = mybir.AluOpType

B, L, CIN = 4, 1024, 20
DM, DS, DC = 128, 64, 4
DI = 256
DTR = 8
DH = 128          # channels per core (d_inner half)
EPS = 1e-5

_cache = {}


def _build():
    nc = bacc.Bacc("TRN2", target_bir_lowering=False, debug=False, num_devices=8)

    # ---- I/O ----
    xt_d = nc.dram_tensor("xt", [CIN, L], F32, kind="ExternalInput")
    wpT_d = nc.dram_tensor("wpT", [CIN, DM], F32, kind="ExternalInput")
    bp_d = nc.dram_tensor("bp", [DM, 1], F32, kind="ExternalInput")
    wiT_d = nc.dram_tensor("wiT", [DM, 3 * DH], F32, kind="ExternalInput")
    convw_d = nc.dram_tensor("convw", [DH, 2 * DC], F32, kind="ExternalInput")
    convb_d = nc.dram_tensor("convb", [DH, 2], F32, kind="ExternalInput")
    wxT_d = nc.dram_tensor("wxT", [DH, 2 * 136], F32, kind="ExternalInput")
    wdtT_d = nc.dram_tensor("wdtT", [DTR, DH], F32, kind="ExternalInput")
    bdt_d = nc.dram_tensor("bdt", [DH, 1], F32, kind="ExternalInput")
    alogp_d = nc.dram_tensor("alogp", [DH, DS], F32, kind="ExternalInput")
    dskip_d = nc.dram_tensor("dskip", [DH, 1], F32, kind="ExternalInput")
    woutT_d = nc.dram_tensor("woutT", [DH, DM], F32, kind="ExternalInput")
    selE_d = nc.dram_tensor("selE", [DH, DS * DH], BF16, kind="ExternalInput")
    pooled_d = nc.dram_tensor("pooled", [DM, 1], F32, kind="ExternalOutput")
    dt_scr = nc.dram_tensor("dt_scr", [DH, L], F32)
    u_scr = nc.dram_tensor("u_scr", [DH, L], BF16)
    bm_scr = nc.dram_tensor("bm_scr", [DS, L], BF16)
    cm_scr = nc.dram_tensor("cm_scr", [DS, L], BF16)

    with tile.TileContext(nc) as tc:
        with (
            tc.tile_pool(name="const", bufs=1) as cp,
            tc.tile_pool(name="work", bufs=1) as wp,
        ):
            # ---- load params ----
            xt = cp.tile([CIN, L], F32)
            wpT = cp.tile([CIN, DM], F32)
            bp = cp.tile([DM, 1], F32)
            wiT = cp.tile([DM, 3 * DH], F32)
            convw = cp.tile([DH, 2 * DC], F32)
            convb = cp.tile([DH, 2], F32)
            wxT = cp.tile([DH, 2 * 136], F32)
            wdtT = cp.tile([DTR, DH], F32)
            bdt = cp.tile([DH, 1], F32)
            alogp = cp.tile([DH, DS], F32)
            dskip = cp.tile([DH, 1], F32)
            woutT = cp.tile([DH, DM], F32)
            selE = cp.tile([DH, DS * DH], BF16)
            for t_, d_ in [(xt, xt_d), (wpT, wpT_d), (bp, bp_d), (wiT, wiT_d),
                           (convw, convw_d), (convb, convb_d), (wxT, wxT_d),
                           (wdtT, wdtT_d), (bdt, bdt_d), (alogp, alogp_d),
                           (dskip, dskip_d), (woutT, woutT_d), (selE, selE_d)]:
                nc.sync.dma_start(t_[:], d_[:])

            HLF = (slice(0, 512), slice(512, 1024))

            # ---- phase 1: front-end ----
            with tc.tile_pool(name="ps1", bufs=4, space="PSUM") as ps1:
                # h = Wp @ x + bp   [128 dm, 1024 t]
                h_ps = ps1.tile([DM, L], F32, tag="ps")
                for sl in HLF:
                    nc.tensor.matmul(h_ps[:, sl], wpT[:, :], xt[:, sl])
                h16 = wp.tile([DM, L], F32)
                nc.scalar.activation(h16[:], h_ps[:], AF.Identity, bias=bp[:])

                # xm_j = W_in[chunk_j] @ h   (j=0 own, j=1 other)
                xmp = []   # padded copies in SBUF
                for j in range(2):
                    xm_ps = ps1.tile([DH, L], F32, tag="ps")
                    for sl in HLF:
                        nc.tensor.matmul(
                            xm_ps[:, sl], wiT[:, j * DH:(j + 1) * DH], h16[:, sl])
                    pad = wp.tile([DH, DC - 1 + L], F32, tag=f"xmp{j}")
                    nc.vector.memset(pad[:, 0:DC - 1], 0.0)
                    nc.scalar.copy(pad[:, DC - 1:DC - 1 + L], xm_ps[:])
                    xmp.append(pad)

                # causal depthwise conv + silu -> xc16_j
                xc16 = []
                for j in range(2):
                    cacc = wp.tile([DH, L], F32, tag=f"cacc{j}")
                    nc.vector.tensor_scalar(
                        out=cacc[:], in0=xmp[j][:, 0:L],
                        scalar1=convw[:, 4 * j:4 * j + 1], scalar2=None,
                        op0=OP.mult)
                    for k in (1, 2, 3):
                        nc.vector.scalar_tensor_tensor(
                            out=cacc[:], in0=xmp[j][:, k:k + L],
                            scalar=convw[:, 4 * j + k:4 * j + k + 1],
                            in1=cacc[:], op0=OP.mult, op1=OP.add)
                    xc = wp.tile([DH, L], F32, tag=f"xc{j}")
                    nc.scalar.activation(xc[:], cacc[:], AF.Silu,
                                         bias=convb[:, j:j + 1])
                    xc16.append(xc)

                # dbc = W_x @ xc  -> dtr [8,L], BmT [64,L], CmT [64,L]
                dtr_ps = ps1.tile([DTR, L], F32, tag="ps")
                bm_ps = ps1.tile([DS, L], F32, tag="ps")
                cm_ps = ps1.tile([DS, L], F32, tag="ps")
                for (m0, msz, out_ps) in ((0, DTR, dtr_ps), (DTR, DS, bm_ps),
                                          (DTR + DS, DS, cm_ps)):
                    for sl in HLF:
                        for j in range(2):
                            nc.tensor.matmul(
                                out_ps[:, sl],
                                wxT[:, 136 * j + m0:136 * j + m0 + msz],
                                xc16[j][:, sl],
                                start=(j == 0), stop=(j == 1))
                dtrT = wp.tile([DTR, L], F32)
                nc.scalar.copy(dtrT[:], dtr_ps[:])
                bmT16 = wp.tile([DS, L], BF16)
                nc.scalar.copy(bmT16[:], bm_ps[:])
                cmT16 = wp.tile([DS, L], BF16)
                nc.scalar.copy(cmT16[:], cm_ps[:])

                # dt = softplus(W_dt @ dtr + b_dt); no Softplus ACT table on
                # TRN2, so compute dt_neg = -dt = ln(sigmoid(-(raw + b_dt)))
                # and pair it with +exp(A_log) in the scan exponentials.
                dt_ps = ps1.tile([DH, L], F32, tag="ps")
                for sl in HLF:
                    nc.tensor.matmul(dt_ps[:, sl], wdtT[:, :], dtrT[:, sl])
                bdtn = wp.tile([DH, 1], F32)
                nc.scalar.mul(bdtn[:], bdt[:], -1.0)
                sg = wp.tile([DH, L], F32)
                nc.scalar.activation(sg[:], dt_ps[:], AF.Sigmoid,
                                     bias=bdtn[:], scale=-1.0)
                DT = wp.tile([DH, L], F32)   # holds -dt
                nc.scalar.activation(DT[:], sg[:], AF.Ln)

            # U = dt * xc_own (bf16) ;  Apos = exp(A_log) in pair layout
            U = wp.tile([DH, L], BF16)
            nc.vector.scalar_tensor_tensor(
                out=U[:], in0=DT[:], scalar=-1.0, in1=xc16[0][:],
                op0=OP.mult, op1=OP.mult)
            aposp = wp.tile([DH, DS], F32)   # +exp(A_log), pairs with -dt
            nc.scalar.activation(aposp[:], alogp[:], AF.Exp)

            # bounce DT/U to DRAM for pair-replication reads
            nc.sync.dma_start(dt_scr[:], DT[:])
            nc.sync.dma_start(u_scr[:], U[:])
            # Bm2/Cm2: [128, L] bf16, partition q = row q//2 of BmT/CmT
            nc.sync.dma_start(bm_scr[:], bmT16[:])
            nc.sync.dma_start(cm_scr[:], cmT16[:])
            Bm2 = wp.tile([DH, L], BF16)
            Cm2 = wp.tile([DH, L], BF16)
            for scr, dst in ((bm_scr, Bm2), (cm_scr, Cm2)):
                sap = scr[:]
                nc.sync.dma_start(dst[:], bass.AP(
                    tensor=sap.tensor, offset=sap.offset,
                    ap=[sap.ap[0], [0, 2], sap.ap[1]]))

            # ---- phase 2: selective scan, pair layout (q = 2n + j) ----
            # pair p covers channels d0=2p, d1=2p+1; partitions hold (n, j)
            NP = DH // 2          # 64 pairs
            GP_HC = 64            # pairs whose HC-mul runs on gpsimd
            with (
                tc.tile_pool(name="psl", bufs=1, space="PSUM") as psl,
                tc.tile_pool(name="sl", bufs=3) as slp,
            ):
                Y_ps = psl.tile([DH, L], F32, tag="Y")
                for p in range(NP):
                    dtrep = slp.tile([DH, L], F32, tag="dtrep")
                    sap = dt_scr[:]
                    nc.sync.dma_start(dtrep[:], bass.AP(
                        tensor=sap.tensor, offset=sap.offset + 2 * p * L,
                        ap=[[0, DS], [L, 2], [1, L]]))
                    urep = slp.tile([DH, L], BF16, tag="urep")
                    sap = u_scr[:]
                    nc.sync.dma_start(urep[:], bass.AP(
                        tensor=sap.tensor, offset=sap.offset + 2 * p * L,
                        ap=[[0, DS], [L, 2], [1, L]]))
                    dAt = slp.tile([DH, L], F32, tag="dA")
                    nc.scalar.activation(dAt[:], dtrep[:], AF.Exp,
                                         scale=aposp[:, p:p + 1])
                    dBxt = slp.tile([DH, L], BF16, tag="dBx")
                    nc.vector.tensor_tensor(out=dBxt[:], in0=urep[:],
                                            in1=Bm2[:], op=OP.mult)
                    Ht = slp.tile([DH, L], BF16, tag="H")
                    nc.vector.tensor_tensor_scan(
                        out=Ht[:], data0=dAt[:], data1=dBxt[:], initial=0.0,
                        op0=OP.mult, op1=OP.add)
                    HCt = slp.tile([DH, L], BF16, tag="HC")
                    eng = nc.gpsimd if p % NP < GP_HC else nc.vector
                    eng.tensor_tensor(out=HCt[:], in0=Ht[:], in1=Cm2[:],
                                      op=OP.mult)
                    selp = selE[:, DH * p:DH * (p + 1)]
                    for sl in HLF:
                        nc.tensor.matmul(Y_ps[:, sl], selp, HCt[:, sl],
                                         start=(p == 0), stop=(p == NP - 1))

            # ---- tail: gate, out-proj, pool ----
                y2 = wp.tile([DH, L], F32)
                nc.vector.scalar_tensor_tensor(
                    out=y2[:], in0=xc16[0][:], scalar=dskip[:], in1=Y_ps[:],
                    op0=OP.mult, op1=OP.add)
            with tc.tile_pool(name="ps2", bufs=2, space="PSUM") as ps2:
                z_ps = ps2.tile([DH, L], F32, tag="z")
                for sl in HLF:
                    nc.tensor.matmul(z_ps[:, sl], wiT[:, 2 * DH:3 * DH],
                                     h16[:, sl])
                zsig = wp.tile([DH, L], F32)
                nc.scalar.activation(zsig[:], z_ps[:], AF.Silu)
                y3 = wp.tile([DH, L], F32)
                nc.vector.tensor_tensor(out=y3[:], in0=y2[:], in1=zsig[:],
                                        op=OP.mult)
                out_ps = ps2.tile([DM, L], F32, tag="o")
                for sl in HLF:
                    nc.tensor.matmul(out_ps[:, sl], woutT[:, :], y3[:, sl])
                trash = wp.tile([DM, L], F32)
                pooled = wp.tile([DM, 1], F32)
                nc.scalar.activation(trash[:], out_ps[:], AF.Identity,
                                     scale=1.0 / L, accum_out=pooled[:])
                nc.sync.dma_start(pooled_d[:], pooled[:])

    nc.compile()
    return nc


def _core_inputs(inputs, b, half):
    f32 = np.float32
    bf16 = ml_dtypes.bfloat16
    x = np.asarray(inputs["x"], f32)
    Wp = np.asarray(inputs["Wp"], f32)
    bp = np.asarray(inputs["bp"], f32)
    W_in = np.asarray(inputs["W_in"], f32)
    conv_w = np.asarray(inputs["conv_w"], f32)
    conv_b = np.asarray(inputs["conv_b"], f32)
    W_x = np.asarray(inputs["W_x"], f32)
    W_dt = np.asarray(inputs["W_dt"], f32)
    b_dt = np.asarray(inputs["b_dt"], f32)
    A_log = np.asarray(inputs["A_log"], f32)
    Dskip = np.asarray(inputs["Dskip"], f32)
    W_out = np.asarray(inputs["W_out"], f32)

    own = slice(half * DH, half * DH + DH)
    other = slice(DH, 2 * DH) if half == 0 else slice(0, DH)
    return {
        "xt": np.ascontiguousarray(x[b]),
        "wpT": np.ascontiguousarray(Wp.T),
        "bp": np.ascontiguousarray(bp[:, None]),
        "wiT": np.concatenate(
            [W_in[0:DI][own].T, W_in[0:DI][other].T,
             W_in[DI:2 * DI][own].T], axis=1),
        "convw": np.concatenate([conv_w[own], conv_w[other]], axis=1),
        "convb": np.stack([conv_b[own], conv_b[other]], axis=1),
        "wxT": np.concatenate([W_x.T[own], W_x.T[other]], axis=1),
        "wdtT": np.ascontiguousarray(W_dt[own].T),
        "bdt": np.ascontiguousarray(b_dt[own][:, None]),
        "alogp": _alog_pairs(A_log[own]),
        "dskip": np.ascontiguousarray(Dskip[own][:, None]),
        "woutT": np.ascontiguousarray(W_out[:, own].T),
        "selE": _selE(),
    }


def _alog_pairs(alog_own):
    # alogp[q, p] = A_log[own][2p + q%2, q//2]
    out = np.empty((DH, DS), np.float32)
    q = np.arange(DH)
    for p in range(DS):
        out[:, p] = alog_own[2 * p + (q % 2), q // 2]
    return out


_selE_cache = {}


def _selE():
    if "v" not in _selE_cache:
        sel = np.zeros((DH, DS * DH), np.float32)
        q = np.arange(DH)
        for p in range(DS):
            sel[q, DH * p + 2 * p + (q % 2)] = 1.0
        _selE_cache["v"] = sel.astype(ml_dtypes.bfloat16)
    return _selE_cache["v"]


def kernel(**inputs) -> np.ndarray:
    if "nc" not in _cache:
        _cache["nc"] = _build()
    nc = _cache["nc"]

    in_maps = [_core_inputs(inputs, c // 2, c % 2) for c in range(8)]
    res = run_bass_kernel_spmd(nc, in_maps, core_ids=list(range(8)))

    pooled = np.zeros((B, DM), np.float32)
    for c in range(8):
        pooled[c // 2] += res.results[c]["pooled"][:, 0]

    # classifier head (host: BatchNorm couples all batches; ~300 flops)
    f32 = np.float32
    W1 = np.asarray(inputs["W1"], f32)
    b1 = np.asarray(inputs["b1"], f32)
    gamma = np.asarray(inputs["gamma"], f32)
    beta = np.asarray(inputs["beta"], f32)
    W2 = np.asarray(inputs["W2"], f32)
    b2 = np.asarray(inputs["b2"], f32)
    h1 = pooled @ W1.T + b1
    mu = h1.mean(axis=0)
    var = h1.var(axis=0)
    h1 = (h1 - mu) / np.sqrt(var + EPS) * gamma + beta
    h1 = np.maximum(h1, 0.0)
    return (h1 @ W2.T + b2).astype(np.float32)


# revision 9
# speedup vs baseline: 1.2032x; 1.1451x over previous
"""Trainium2 Bass kernel for nn_BASE_MAMBA_14018773254552.

Mamba block (d_model=128, d_inner=256, d_state=64, d_conv=4, L=1024, B=4)
+ input proj + classifier head.

Sharding: 8 cores = 4 batches x 2 d_inner-halves (128 channels each).
Each core computes its batch's full front-end (input proj, in_proj, conv,
x_proj) feature-major ([feature, time] tiles), then the selective scan for
its 128-channel half, and the partial out-proj + mean-pool. The host sums
the two channel-half partials per batch and runs the tiny classifier
(BatchNorm couples batches, so it cannot live on one core).

Self-contained: hardcodes all shapes; builds + compiles the Bass program
once per process and runs it on cores 0-7 via run_bass_kernel_spmd.
"""
import numpy as np

try:
    import concourse.bacc as bacc
except ImportError:  # pragma: no cover - path fallback
    import sys
    for _p in ("/opt/trn_rl_repo", "/root/.axon_site/_ro/trn_rl_repo"):
        if _p not in sys.path:
            sys.path.insert(0, _p)
    import concourse.bacc as bacc

import ml_dtypes
import concourse.bass as bass
import concourse.mybir as mybir
import concourse.tile as tile
from concourse.bass_utils import run_bass_kernel_spmd

F32 = mybir.dt.float32
BF16 = mybir.dt.bfloat16
AF = mybir.ActivationFunctionType
OP = mybir.AluOpType

B, L, CIN = 4, 1024, 20
DM, DS, DC = 128, 64, 4
DI = 256
DTR = 8
DH = 128          # channels per core (d_inner half)
EPS = 1e-5

_cache = {}


def _build():
    nc = bacc.Bacc("TRN2", target_bir_lowering=False, debug=False, num_devices=8)

    # ---- I/O ----
    xt_d = nc.dram_tensor("xt", [CIN, L], F32, kind="ExternalInput")
    wpT_d = nc.dram_tensor("wpT", [CIN, DM], F32, kind="ExternalInput")
    bp_d = nc.dram_tensor("bp", [DM, 1], F32, kind="ExternalInput")
    wiT_d = nc.dram_tensor("wiT", [DM, 3 * DH], F32, kind="ExternalInput")
    convw_d = nc.dram_tensor("convw", [DH, 2 * DC], F32, kind="ExternalInput")
    convb_d = nc.dram_tensor("convb", [DH, 2], F32, kind="ExternalInput")
    wxT_d = nc.dram_tensor("wxT", [DH, 2 * 136], F32, kind="ExternalInput")
    wdtT_d = nc.dram_tensor("wdtT", [DTR, DH], F32, kind="ExternalInput")
    bdt_d = nc.dram_tensor("bdt", [DH, 1], F32, kind="ExternalInput")
    alogp_d = nc.dram_tensor("alogp", [DH, DS], F32, kind="ExternalInput")
    dskip_d = nc.dram_tensor("dskip", [DH, 1], F32, kind="ExternalInput")
    woutT_d = nc.dram_tensor("woutT", [DH, DM], F32, kind="ExternalInput")
    selE_d = nc.dram_tensor("selE", [DH, DS * DH], BF16, kind="ExternalInput")
    pooled_d = nc.dram_tensor("pooled", [DM, 1], F32, kind="ExternalOutput")
    dt_scr = nc.dram_tensor("dt_scr", [DH, L], BF16)
    u_scr = nc.dram_tensor("u_scr", [DH, L], BF16)
    bm_scr = nc.dram_tensor("bm_scr", [DS, L], BF16)
    cm_scr = nc.dram_tensor("cm_scr", [DS, L], BF16)

    with tile.TileContext(nc) as tc:
        with (
            tc.tile_pool(name="const", bufs=1) as cp,
            tc.tile_pool(name="work", bufs=1) as wp,
        ):
            # ---- load params ----
            xt = cp.tile([CIN, L], F32)
            wpT = cp.tile([CIN, DM], F32)
            bp = cp.tile([DM, 1], F32)
            wiT = cp.tile([DM, 3 * DH], F32)
            convw = cp.tile([DH, 2 * DC], F32)
            convb = cp.tile([DH, 2], F32)
            wxT = cp.tile([DH, 2 * 136], F32)
            wdtT = cp.tile([DTR, DH], F32)
            bdt = cp.tile([DH, 1], F32)
            alogp = cp.tile([DH, DS], F32)
            dskip = cp.tile([DH, 1], F32)
            woutT = cp.tile([DH, DM], F32)
            selE = cp.tile([DH, DS * DH], BF16)
            for t_, d_ in [(xt, xt_d), (wpT, wpT_d), (bp, bp_d), (wiT, wiT_d),
                           (convw, convw_d), (convb, convb_d), (wxT, wxT_d),
                           (wdtT, wdtT_d), (bdt, bdt_d), (alogp, alogp_d),
                           (dskip, dskip_d), (woutT, woutT_d), (selE, selE_d)]:
                nc.sync.dma_start(t_[:], d_[:])

            HLF = (slice(0, 512), slice(512, 1024))

            # ---- phase 1: front-end ----
            with tc.tile_pool(name="ps1", bufs=4, space="PSUM") as ps1:
                # h = Wp @ x + bp   [128 dm, 1024 t]
                h_ps = ps1.tile([DM, L], F32, tag="ps")
                for sl in HLF:
                    nc.tensor.matmul(h_ps[:, sl], wpT[:, :], xt[:, sl])
                h16 = wp.tile([DM, L], F32)
                nc.scalar.activation(h16[:], h_ps[:], AF.Identity, bias=bp[:])

                # xm_j = W_in[chunk_j] @ h   (j=0 own, j=1 other)
                xmp = []   # padded copies in SBUF
                for j in range(2):
                    xm_ps = ps1.tile([DH, L], F32, tag="ps")
                    for sl in HLF:
                        nc.tensor.matmul(
                            xm_ps[:, sl], wiT[:, j * DH:(j + 1) * DH], h16[:, sl])
                    pad = wp.tile([DH, DC - 1 + L], F32, tag=f"xmp{j}")
                    nc.vector.memset(pad[:, 0:DC - 1], 0.0)
                    nc.scalar.copy(pad[:, DC - 1:DC - 1 + L], xm_ps[:])
                    xmp.append(pad)

                # causal depthwise conv + silu -> xc16_j
                xc16 = []
                for j in range(2):
                    cacc = wp.tile([DH, L], F32, tag=f"cacc{j}")
                    nc.vector.tensor_scalar(
                        out=cacc[:], in0=xmp[j][:, 0:L],
                        scalar1=convw[:, 4 * j:4 * j + 1], scalar2=None,
                        op0=OP.mult)
                    for k in (1, 2, 3):
                        nc.vector.scalar_tensor_tensor(
                            out=cacc[:], in0=xmp[j][:, k:k + L],
                            scalar=convw[:, 4 * j + k:4 * j + k + 1],
                            in1=cacc[:], op0=OP.mult, op1=OP.add)
                    xc = wp.tile([DH, L], F32, tag=f"xc{j}")
                    nc.scalar.activation(xc[:], cacc[:], AF.Silu,
                                         bias=convb[:, j:j + 1])
                    xc16.append(xc)

                # dbc = W_x @ xc  -> dtr [8,L], BmT [64,L], CmT [64,L]
                dtr_ps = ps1.tile([DTR, L], F32, tag="ps")
                bm_ps = ps1.tile([DS, L], F32, tag="ps")
                cm_ps = ps1.tile([DS, L], F32, tag="ps")
                for (m0, msz, out_ps) in ((0, DTR, dtr_ps), (DTR, DS, bm_ps),
                                          (DTR + DS, DS, cm_ps)):
                    for sl in HLF:
                        for j in range(2):
                            nc.tensor.matmul(
                                out_ps[:, sl],
                                wxT[:, 136 * j + m0:136 * j + m0 + msz],
                                xc16[j][:, sl],
                                start=(j == 0), stop=(j == 1))
                dtrT = wp.tile([DTR, L], F32)
                nc.scalar.copy(dtrT[:], dtr_ps[:])
                bmT16 = wp.tile([DS, L], BF16)
                nc.scalar.copy(bmT16[:], bm_ps[:])
                cmT16 = wp.tile([DS, L], BF16)
                nc.scalar.copy(cmT16[:], cm_ps[:])

                # dt = softplus(W_dt @ dtr + b_dt); no Softplus ACT table on
                # TRN2, so compute dt_neg = -dt = ln(sigmoid(-(raw + b_dt)))
                # and pair it with +exp(A_log) in the scan exponentials.
                dt_ps = ps1.tile([DH, L], F32, tag="ps")
                for sl in HLF:
                    nc.tensor.matmul(dt_ps[:, sl], wdtT[:, :], dtrT[:, sl])
                bdtn = wp.tile([DH, 1], F32)
                nc.scalar.mul(bdtn[:], bdt[:], -1.0)
                sg = wp.tile([DH, L], F32)
                nc.scalar.activation(sg[:], dt_ps[:], AF.Sigmoid,
                                     bias=bdtn[:], scale=-1.0)
                DT = wp.tile([DH, L], BF16)  # holds -dt
                nc.scalar.activation(DT[:], sg[:], AF.Ln)

            # U = dt * xc_own (bf16) ;  Apos = exp(A_log) in pair layout
            U = wp.tile([DH, L], BF16)
            nc.vector.scalar_tensor_tensor(
                out=U[:], in0=DT[:], scalar=-1.0, in1=xc16[0][:],
                op0=OP.mult, op1=OP.mult)
            aposp = wp.tile([DH, DS], F32)   # +exp(A_log), pairs with -dt
            nc.scalar.activation(aposp[:], alogp[:], AF.Exp)

            # bounce DT/U to DRAM for pair-replication reads
            nc.sync.dma_start(dt_scr[:], DT[:])
            nc.sync.dma_start(u_scr[:], U[:])
            # Bm2/Cm2: [128, L] bf16, partition q = row q//2 of BmT/CmT
            nc.sync.dma_start(bm_scr[:], bmT16[:])
            nc.sync.dma_start(cm_scr[:], cmT16[:])
            Bm2 = wp.tile([DH, L], BF16)
            Cm2 = wp.tile([DH, L], BF16)
            for scr, dst in ((bm_scr, Bm2), (cm_scr, Cm2)):
                sap = scr[:]
                nc.sync.dma_start(dst[:], bass.AP(
                    tensor=sap.tensor, offset=sap.offset,
                    ap=[sap.ap[0], [0, 2], sap.ap[1]]))

            # ---- phase 2: selective scan, pair layout (q = 2n + j) ----
            # pair p covers channels d0=2p, d1=2p+1; partitions hold (n, j)
            NP = DH // 2          # 64 pairs
            GP_HC = 7             # of every 16 pairs, this many HC on gpsimd
            with (
                tc.tile_pool(name="psl", bufs=1, space="PSUM") as psl,
                tc.tile_pool(name="sl", bufs=5) as slp,
            ):
                Y_ps = psl.tile([DH, L], F32, tag="Y")
                for p in range(NP):
                    dtrep = slp.tile([DH, L], BF16, tag="dtrep")
                    sap = dt_scr[:]
                    nc.sync.dma_start(dtrep[:], bass.AP(
                        tensor=sap.tensor, offset=sap.offset + 2 * p * L,
                        ap=[[0, DS], [L, 2], [1, L]]))
                    urep = slp.tile([DH, L], BF16, tag="urep")
                    sap = u_scr[:]
                    nc.sync.dma_start(urep[:], bass.AP(
                        tensor=sap.tensor, offset=sap.offset + 2 * p * L,
                        ap=[[0, DS], [L, 2], [1, L]]))
                    dAt = slp.tile([DH, L], F32, tag="dA")
                    nc.scalar.activation(dAt[:], dtrep[:], AF.Exp,
                                         scale=aposp[:, p:p + 1])
                    dBxt = slp.tile([DH, L], BF16, tag="dBx")
                    nc.vector.tensor_tensor(out=dBxt[:], in0=urep[:],
                                            in1=Bm2[:], op=OP.mult)
                    Ht = slp.tile([DH, L], BF16, tag="H")
                    nc.vector.tensor_tensor_scan(
                        out=Ht[:], data0=dAt[:], data1=dBxt[:], initial=0.0,
                        op0=OP.mult, op1=OP.add)
                    HCt = slp.tile([DH, L], BF16, tag="HC")
                    eng = nc.gpsimd if (p % 16) < GP_HC else nc.vector
                    eng.tensor_tensor(out=HCt[:], in0=Ht[:], in1=Cm2[:],
                                      op=OP.mult)
                    selp = selE[:, DH * p:DH * (p + 1)]
                    for sl in HLF:
                        nc.tensor.matmul(Y_ps[:, sl], selp, HCt[:, sl],
                                         start=(p == 0), stop=(p == NP - 1))

            # ---- tail: gate, out-proj, pool ----
                y2 = wp.tile([DH, L], F32)
                nc.vector.scalar_tensor_tensor(
                    out=y2[:], in0=xc16[0][:], scalar=dskip[:], in1=Y_ps[:],
                    op0=OP.mult, op1=OP.add)
            with tc.tile_pool(name="ps2", bufs=2, space="PSUM") as ps2:
                z_ps = ps2.tile([DH, L], F32, tag="z")
                for sl in HLF:
                    nc.tensor.matmul(z_ps[:, sl], wiT[:, 2 * DH:3 * DH],
                                     h16[:, sl])
                zsig = wp.tile([DH, L], F32)
                nc.scalar.activation(zsig[:], z_ps[:], AF.Silu)
                y3 = wp.tile([DH, L], F32)
                nc.vector.tensor_tensor(out=y3[:], in0=y2[:], in1=zsig[:],
                                        op=OP.mult)
                out_ps = ps2.tile([DM, L], F32, tag="o")
                for sl in HLF:
                    nc.tensor.matmul(out_ps[:, sl], woutT[:, :], y3[:, sl])
                trash = wp.tile([DM, L], F32)
                pooled = wp.tile([DM, 1], F32)
                nc.scalar.activation(trash[:], out_ps[:], AF.Identity,
                                     scale=1.0 / L, accum_out=pooled[:])
                nc.sync.dma_start(pooled_d[:], pooled[:])

    nc.compile()
    return nc


def _core_inputs(inputs, b, half):
    f32 = np.float32
    bf16 = ml_dtypes.bfloat16
    x = np.asarray(inputs["x"], f32)
    Wp = np.asarray(inputs["Wp"], f32)
    bp = np.asarray(inputs["bp"], f32)
    W_in = np.asarray(inputs["W_in"], f32)
    conv_w = np.asarray(inputs["conv_w"], f32)
    conv_b = np.asarray(inputs["conv_b"], f32)
    W_x = np.asarray(inputs["W_x"], f32)
    W_dt = np.asarray(inputs["W_dt"], f32)
    b_dt = np.asarray(inputs["b_dt"], f32)
    A_log = np.asarray(inputs["A_log"], f32)
    Dskip = np.asarray(inputs["Dskip"], f32)
    W_out = np.asarray(inputs["W_out"], f32)

    own = slice(half * DH, half * DH + DH)
    other = slice(DH, 2 * DH) if half == 0 else slice(0, DH)
    return {
        "xt": np.ascontiguousarray(x[b]),
        "wpT": np.ascontiguousarray(Wp.T),
        "bp": np.ascontiguousarray(bp[:, None]),
        "wiT": np.concatenate(
            [W_in[0:DI][own].T, W_in[0:DI][other].T,
             W_in[DI:2 * DI][own].T], axis=1),
        "convw": np.concatenate([conv_w[own], conv_w[other]], axis=1),
        "convb": np.stack([conv_b[own], conv_b[other]], axis=1),
        "wxT": np.concatenate([W_x.T[own], W_x.T[other]], axis=1),
        "wdtT": np.ascontiguousarray(W_dt[own].T),
        "bdt": np.ascontiguousarray(b_dt[own][:, None]),
        "alogp": _alog_pairs(A_log[own]),
        "dskip": np.ascontiguousarray(Dskip[own][:, None]),
        "woutT": np.ascontiguousarray(W_out[:, own].T),
        "selE": _selE(),
    }


def _alog_pairs(alog_own):
    # alogp[q, p] = A_log[own][2p + q%2, q//2]
    out = np.empty((DH, DS), np.float32)
    q = np.arange(DH)
    for p in range(DS):
        out[:, p] = alog_own[2 * p + (q % 2), q // 2]
    return out


_selE_cache = {}


def _selE():
    if "v" not in _selE_cache:
        sel = np.zeros((DH, DS * DH), np.float32)
        q = np.arange(DH)
        for p in range(DS):
            sel[q, DH * p + 2 * p + (q % 2)] = 1.0
        _selE_cache["v"] = sel.astype(ml_dtypes.bfloat16)
    return _selE_cache["v"]


def kernel(**inputs) -> np.ndarray:
    if "nc" not in _cache:
        _cache["nc"] = _build()
    nc = _cache["nc"]

    in_maps = [_core_inputs(inputs, c // 2, c % 2) for c in range(8)]
    res = run_bass_kernel_spmd(nc, in_maps, core_ids=list(range(8)))

    pooled = np.zeros((B, DM), np.float32)
    for c in range(8):
        pooled[c // 2] += res.results[c]["pooled"][:, 0]

    # classifier head (host: BatchNorm couples all batches; ~300 flops)
    f32 = np.float32
    W1 = np.asarray(inputs["W1"], f32)
    b1 = np.asarray(inputs["b1"], f32)
    gamma = np.asarray(inputs["gamma"], f32)
    beta = np.asarray(inputs["beta"], f32)
    W2 = np.asarray(inputs["W2"], f32)
    b2 = np.asarray(inputs["b2"], f32)
    h1 = pooled @ W1.T + b1
    mu = h1.mean(axis=0)
    var = h1.var(axis=0)
    h1 = (h1 - mu) / np.sqrt(var + EPS) * gamma + beta
    h1 = np.maximum(h1, 0.0)
    return (h1 @ W2.T + b2).astype(np.float32)


# revision 10
# speedup vs baseline: 1.2129x; 1.0081x over previous
"""Trainium2 Bass kernel for nn_BASE_MAMBA_14018773254552.

Mamba block (d_model=128, d_inner=256, d_state=64, d_conv=4, L=1024, B=4)
+ input proj + classifier head.

Sharding: 8 cores = 4 batches x 2 d_inner-halves (128 channels each).
Each core computes its batch's full front-end (input proj, in_proj, conv,
x_proj) feature-major ([feature, time] tiles), then the selective scan for
its 128-channel half, and the partial out-proj + mean-pool. The host sums
the two channel-half partials per batch and runs the tiny classifier
(BatchNorm couples batches, so it cannot live on one core).

Self-contained: hardcodes all shapes; builds + compiles the Bass program
once per process and runs it on cores 0-7 via run_bass_kernel_spmd.
"""
import numpy as np

try:
    import concourse.bacc as bacc
except ImportError:  # pragma: no cover - path fallback
    import sys
    for _p in ("/opt/trn_rl_repo", "/root/.axon_site/_ro/trn_rl_repo"):
        if _p not in sys.path:
            sys.path.insert(0, _p)
    import concourse.bacc as bacc

import ml_dtypes
import concourse.bass as bass
import concourse.mybir as mybir
import concourse.tile as tile
from concourse.bass_utils import run_bass_kernel_spmd

F32 = mybir.dt.float32
BF16 = mybir.dt.bfloat16
AF = mybir.ActivationFunctionType
OP = mybir.AluOpType

B, L, CIN = 4, 1024, 20
DM, DS, DC = 128, 64, 4
DI = 256
DTR = 8
DH = 128          # channels per core (d_inner half)
EPS = 1e-5

_cache = {}


def _build():
    nc = bacc.Bacc("TRN2", target_bir_lowering=False, debug=False, num_devices=8)

    # ---- I/O ----
    xt_d = nc.dram_tensor("xt", [CIN, L], F32, kind="ExternalInput")
    wpT_d = nc.dram_tensor("wpT", [CIN, DM], F32, kind="ExternalInput")
    bp_d = nc.dram_tensor("bp", [DM, 1], F32, kind="ExternalInput")
    wiT_d = nc.dram_tensor("wiT", [DM, 3 * DH], F32, kind="ExternalInput")
    convw_d = nc.dram_tensor("convw", [DH, 2 * DC], F32, kind="ExternalInput")
    convb_d = nc.dram_tensor("convb", [DH, 2], F32, kind="ExternalInput")
    wxT_d = nc.dram_tensor("wxT", [DH, 2 * 136], F32, kind="ExternalInput")
    wdtT_d = nc.dram_tensor("wdtT", [DTR, DH], F32, kind="ExternalInput")
    bdt_d = nc.dram_tensor("bdt", [DH, 1], F32, kind="ExternalInput")
    alogp_d = nc.dram_tensor("alogp", [DH, DS], F32, kind="ExternalInput")
    dskip_d = nc.dram_tensor("dskip", [DH, 1], F32, kind="ExternalInput")
    woutT_d = nc.dram_tensor("woutT", [DH, DM], F32, kind="ExternalInput")
    selE_d = nc.dram_tensor("selE", [DH, DS * DH], BF16, kind="ExternalInput")
    pooled_d = nc.dram_tensor("pooled", [DM, 1], F32, kind="ExternalOutput")
    dt_scr = nc.dram_tensor("dt_scr", [DH, L], BF16)
    u_scr = nc.dram_tensor("u_scr", [DH, L], BF16)
    bm_scr = nc.dram_tensor("bm_scr", [DS, L], BF16)
    cm_scr = nc.dram_tensor("cm_scr", [DS, L], BF16)

    with tile.TileContext(nc) as tc:
        with (
            tc.tile_pool(name="const", bufs=1) as cp,
            tc.tile_pool(name="work", bufs=1) as wp,
        ):
            # ---- load params ----
            xt = cp.tile([CIN, L], F32)
            wpT = cp.tile([CIN, DM], F32)
            bp = cp.tile([DM, 1], F32)
            wiT = cp.tile([DM, 3 * DH], F32)
            convw = cp.tile([DH, 2 * DC], F32)
            convb = cp.tile([DH, 2], F32)
            wxT = cp.tile([DH, 2 * 136], F32)
            wdtT = cp.tile([DTR, DH], F32)
            bdt = cp.tile([DH, 1], F32)
            alogp = cp.tile([DH, DS], F32)
            dskip = cp.tile([DH, 1], F32)
            woutT = cp.tile([DH, DM], F32)
            selE = cp.tile([DH, DS * DH], BF16)
            for t_, d_ in [(xt, xt_d), (wpT, wpT_d), (bp, bp_d), (wiT, wiT_d),
                           (convw, convw_d), (convb, convb_d), (wxT, wxT_d),
                           (wdtT, wdtT_d), (bdt, bdt_d), (alogp, alogp_d),
                           (dskip, dskip_d), (woutT, woutT_d), (selE, selE_d)]:
                nc.sync.dma_start(t_[:], d_[:])

            HLF = (slice(0, 512), slice(512, 1024))

            # ---- phase 1: front-end ----
            with tc.tile_pool(name="ps1", bufs=4, space="PSUM") as ps1:
                # h = Wp @ x + bp   [128 dm, 1024 t]
                h_ps = ps1.tile([DM, L], F32, tag="ps")
                for sl in HLF:
                    nc.tensor.matmul(h_ps[:, sl], wpT[:, :], xt[:, sl])
                h16 = wp.tile([DM, L], F32)
                nc.scalar.activation(h16[:], h_ps[:], AF.Identity, bias=bp[:])

                # xm_j = W_in[chunk_j] @ h   (j=0 own, j=1 other)
                xmp = []   # padded copies in SBUF
                for j in range(2):
                    xm_ps = ps1.tile([DH, L], F32, tag="ps")
                    for sl in HLF:
                        nc.tensor.matmul(
                            xm_ps[:, sl], wiT[:, j * DH:(j + 1) * DH], h16[:, sl])
                    pad = wp.tile([DH, DC - 1 + L], F32, tag=f"xmp{j}")
                    nc.vector.memset(pad[:, 0:DC - 1], 0.0)
                    nc.scalar.copy(pad[:, DC - 1:DC - 1 + L], xm_ps[:])
                    xmp.append(pad)

                # causal depthwise conv + silu -> xc16_j
                xc16 = []
                for j in range(2):
                    c01 = wp.tile([DH, L], F32, tag=f"c01_{j}")
                    nc.vector.tensor_scalar(
                        out=c01[:], in0=xmp[j][:, 0:L],
                        scalar1=convw[:, 4 * j:4 * j + 1], scalar2=None,
                        op0=OP.mult)
                    nc.vector.scalar_tensor_tensor(
                        out=c01[:], in0=xmp[j][:, 1:1 + L],
                        scalar=convw[:, 4 * j + 1:4 * j + 2],
                        in1=c01[:], op0=OP.mult, op1=OP.add)
                    c23 = wp.tile([DH, L], F32, tag=f"c23_{j}")
                    nc.vector.tensor_scalar(
                        out=c23[:], in0=xmp[j][:, 2:2 + L],
                        scalar1=convw[:, 4 * j + 2:4 * j + 3], scalar2=None,
                        op0=OP.mult)
                    nc.vector.scalar_tensor_tensor(
                        out=c23[:], in0=xmp[j][:, 3:3 + L],
                        scalar=convw[:, 4 * j + 3:4 * j + 4],
                        in1=c23[:], op0=OP.mult, op1=OP.add)
                    cacc = wp.tile([DH, L], F32, tag=f"cacc{j}")
                    nc.vector.tensor_tensor(out=cacc[:], in0=c01[:],
                                            in1=c23[:], op=OP.add)
                    xc = wp.tile([DH, L], F32, tag=f"xc{j}")
                    nc.scalar.activation(xc[:], cacc[:], AF.Silu,
                                         bias=convb[:, j:j + 1])
                    xc16.append(xc)

                # dbc = W_x @ xc  -> dtr [8,L], BmT [64,L], CmT [64,L]
                dtr_ps = ps1.tile([DTR, L], F32, tag="ps")
                bm_ps = ps1.tile([DS, L], F32, tag="ps")
                cm_ps = ps1.tile([DS, L], F32, tag="ps")
                for (m0, msz, out_ps) in ((0, DTR, dtr_ps), (DTR, DS, bm_ps),
                                          (DTR + DS, DS, cm_ps)):
                    for sl in HLF:
                        for j in range(2):
                            nc.tensor.matmul(
                                out_ps[:, sl],
                                wxT[:, 136 * j + m0:136 * j + m0 + msz],
                                xc16[j][:, sl],
                                start=(j == 0), stop=(j == 1))
                dtrT = wp.tile([DTR, L], F32)
                nc.scalar.copy(dtrT[:], dtr_ps[:])
                bmT16 = wp.tile([DS, L], BF16)
                nc.scalar.copy(bmT16[:], bm_ps[:])
                cmT16 = wp.tile([DS, L], BF16)
                nc.scalar.copy(cmT16[:], cm_ps[:])

                # dt = softplus(W_dt @ dtr + b_dt); no Softplus ACT table on
                # TRN2, so compute dt_neg = -dt = ln(sigmoid(-(raw + b_dt)))
                # and pair it with +exp(A_log) in the scan exponentials.
                dt_ps = ps1.tile([DH, L], F32, tag="ps")
                for sl in HLF:
                    nc.tensor.matmul(dt_ps[:, sl], wdtT[:, :], dtrT[:, sl])
                bdtn = wp.tile([DH, 1], F32)
                nc.scalar.mul(bdtn[:], bdt[:], -1.0)
                sg = wp.tile([DH, L], F32)
                nc.scalar.activation(sg[:], dt_ps[:], AF.Sigmoid,
                                     bias=bdtn[:], scale=-1.0)
                DT = wp.tile([DH, L], BF16)  # holds -dt
                nc.scalar.activation(DT[:], sg[:], AF.Ln)

            # U = dt * xc_own (bf16) ;  Apos = exp(A_log) in pair layout
            U = wp.tile([DH, L], BF16)
            nc.vector.scalar_tensor_tensor(
                out=U[:], in0=DT[:], scalar=-1.0, in1=xc16[0][:],
                op0=OP.mult, op1=OP.mult)
            aposp = wp.tile([DH, DS], F32)   # +exp(A_log), pairs with -dt
            nc.scalar.activation(aposp[:], alogp[:], AF.Exp)

            # bounce DT/U to DRAM for pair-replication reads
            nc.sync.dma_start(dt_scr[:], DT[:])
            nc.sync.dma_start(u_scr[:], U[:])
            # Bm2/Cm2: [128, L] bf16, partition q = row q//2 of BmT/CmT
            nc.sync.dma_start(bm_scr[:], bmT16[:])
            nc.sync.dma_start(cm_scr[:], cmT16[:])
            Bm2 = wp.tile([DH, L], BF16)
            Cm2 = wp.tile([DH, L], BF16)
            for scr, dst in ((bm_scr, Bm2), (cm_scr, Cm2)):
                sap = scr[:]
                nc.sync.dma_start(dst[:], bass.AP(
                    tensor=sap.tensor, offset=sap.offset,
                    ap=[sap.ap[0], [0, 2], sap.ap[1]]))

            # z-gate early (PE/ACT idle-ish here; frees the tail)
            zsig = wp.tile([DH, L], F32)
            with tc.tile_pool(name="psz", bufs=1, space="PSUM") as psz:
                z_ps = psz.tile([DH, L], F32, tag="z")
                for sl in HLF:
                    nc.tensor.matmul(z_ps[:, sl], wiT[:, 2 * DH:3 * DH],
                                     h16[:, sl])
                nc.scalar.activation(zsig[:], z_ps[:], AF.Silu)

            # ---- phase 2: selective scan, pair layout (q = 2n + j) ----
            # pair p covers channels d0=2p, d1=2p+1; partitions hold (n, j)
            NP = DH // 2          # 64 pairs
            GP_HC = 7             # of every 16 pairs, this many HC on gpsimd
            with (
                tc.tile_pool(name="psl", bufs=1, space="PSUM") as psl,
                tc.tile_pool(name="sl", bufs=5) as slp,
            ):
                Y_ps = psl.tile([DH, L], F32, tag="Y")
                for p in range(NP):
                    dtrep = slp.tile([DH, L], BF16, tag="dtrep")
                    sap = dt_scr[:]
                    nc.sync.dma_start(dtrep[:], bass.AP(
                        tensor=sap.tensor, offset=sap.offset + 2 * p * L,
                        ap=[[0, DS], [L, 2], [1, L]]))
                    urep = slp.tile([DH, L], BF16, tag="urep")
                    sap = u_scr[:]
                    nc.sync.dma_start(urep[:], bass.AP(
                        tensor=sap.tensor, offset=sap.offset + 2 * p * L,
                        ap=[[0, DS], [L, 2], [1, L]]))
                    dAt = slp.tile([DH, L], F32, tag="dA")
                    nc.scalar.activation(dAt[:], dtrep[:], AF.Exp,
                                         scale=aposp[:, p:p + 1])
                    dBxt = slp.tile([DH, L], BF16, tag="dBx")
                    nc.vector.tensor_tensor(out=dBxt[:], in0=urep[:],
                                            in1=Bm2[:], op=OP.mult)
                    Ht = slp.tile([DH, L], BF16, tag="H")
                    nc.vector.tensor_tensor_scan(
                        out=Ht[:], data0=dAt[:], data1=dBxt[:], initial=0.0,
                        op0=OP.mult, op1=OP.add)
                    HCt = slp.tile([DH, L], BF16, tag="HC")
                    eng = nc.gpsimd if (p % 16) < GP_HC else nc.vector
                    eng.tensor_tensor(out=HCt[:], in0=Ht[:], in1=Cm2[:],
                                      op=OP.mult)
                    selp = selE[:, DH * p:DH * (p + 1)]
                    for sl in HLF:
                        nc.tensor.matmul(Y_ps[:, sl], selp, HCt[:, sl],
                                         start=(p == 0), stop=(p == NP - 1))

            # ---- tail: gate, out-proj, pool (pipelined by t-half) ----
                y2 = wp.tile([DH, L], F32)
                y3 = wp.tile([DH, L], F32)
                trash = wp.tile([DM, L], F32)
                pooled_h = wp.tile([DM, 2], F32)
                pooled = wp.tile([DM, 1], F32)
                with tc.tile_pool(name="ps2", bufs=2, space="PSUM") as ps2:
                    out_ps = ps2.tile([DM, L], F32, tag="o")
                    for hi, sl in enumerate(HLF):
                        nc.vector.scalar_tensor_tensor(
                            out=y2[:, sl], in0=xc16[0][:, sl], scalar=dskip[:],
                            in1=Y_ps[:, sl], op0=OP.mult, op1=OP.add)
                        nc.vector.tensor_tensor(out=y3[:, sl], in0=y2[:, sl],
                                                in1=zsig[:, sl], op=OP.mult)
                        nc.tensor.matmul(out_ps[:, sl], woutT[:, :], y3[:, sl])
                        nc.scalar.activation(
                            trash[:, sl], out_ps[:, sl], AF.Identity,
                            scale=1.0 / L, accum_out=pooled_h[:, hi:hi + 1])
                    nc.vector.tensor_tensor(
                        out=pooled[:], in0=pooled_h[:, 0:1],
                        in1=pooled_h[:, 1:2], op=OP.add)
                    nc.sync.dma_start(pooled_d[:], pooled[:])

    nc.compile()
    return nc


def _core_inputs(inputs, b, half):
    f32 = np.float32
    bf16 = ml_dtypes.bfloat16
    x = np.asarray(inputs["x"], f32)
    Wp = np.asarray(inputs["Wp"], f32)
    bp = np.asarray(inputs["bp"], f32)
    W_in = np.asarray(inputs["W_in"], f32)
    conv_w = np.asarray(inputs["conv_w"], f32)
    conv_b = np.asarray(inputs["conv_b"], f32)
    W_x = np.asarray(inputs["W_x"], f32)
    W_dt = np.asarray(inputs["W_dt"], f32)
    b_dt = np.asarray(inputs["b_dt"], f32)
    A_log = np.asarray(inputs["A_log"], f32)
    Dskip = np.asarray(inputs["Dskip"], f32)
    W_out = np.asarray(inputs["W_out"], f32)

    own = slice(half * DH, half * DH + DH)
    other = slice(DH, 2 * DH) if half == 0 else slice(0, DH)
    return {
        "xt": np.ascontiguousarray(x[b]),
        "wpT": np.ascontiguousarray(Wp.T),
        "bp": np.ascontiguousarray(bp[:, None]),
        "wiT": np.concatenate(
            [W_in[0:DI][own].T, W_in[0:DI][other].T,
             W_in[DI:2 * DI][own].T], axis=1),
        "convw": np.concatenate([conv_w[own], conv_w[other]], axis=1),
        "convb": np.stack([conv_b[own], conv_b[other]], axis=1),
        "wxT": np.concatenate([W_x.T[own], W_x.T[other]], axis=1),
        "wdtT": np.ascontiguousarray(W_dt[own].T),
        "bdt": np.ascontiguousarray(b_dt[own][:, None]),
        "alogp": _alog_pairs(A_log[own]),
        "dskip": np.ascontiguousarray(Dskip[own][:, None]),
        "woutT": np.ascontiguousarray(W_out[:, own].T),
        "selE": _selE(),
    }


def _alog_pairs(alog_own):
    # alogp[q, p] = A_log[own][2p + q%2, q//2]
    out = np.empty((DH, DS), np.float32)
    q = np.arange(DH)
    for p in range(DS):
        out[:, p] = alog_own[2 * p + (q % 2), q // 2]
    return out


_selE_cache = {}


def _selE():
    if "v" not in _selE_cache:
        sel = np.zeros((DH, DS * DH), np.float32)
        q = np.arange(DH)
        for p in range(DS):
            sel[q, DH * p + 2 * p + (q % 2)] = 1.0
        _selE_cache["v"] = sel.astype(ml_dtypes.bfloat16)
    return _selE_cache["v"]


def kernel(**inputs) -> np.ndarray:
    if "nc" not in _cache:
        _cache["nc"] = _build()
    nc = _cache["nc"]

    in_maps = [_core_inputs(inputs, c // 2, c % 2) for c in range(8)]
    res = run_bass_kernel_spmd(nc, in_maps, core_ids=list(range(8)))

    pooled = np.zeros((B, DM), np.float32)
    for c in range(8):
        pooled[c // 2] += res.results[c]["pooled"][:, 0]

    # classifier head (host: BatchNorm couples all batches; ~300 flops)
    f32 = np.float32
    W1 = np.asarray(inputs["W1"], f32)
    b1 = np.asarray(inputs["b1"], f32)
    gamma = np.asarray(inputs["gamma"], f32)
    beta = np.asarray(inputs["beta"], f32)
    W2 = np.asarray(inputs["W2"], f32)
    b2 = np.asarray(inputs["b2"], f32)
    h1 = pooled @ W1.T + b1
    mu = h1.mean(axis=0)
    var = h1.var(axis=0)
    h1 = (h1 - mu) / np.sqrt(var + EPS) * gamma + beta
    h1 = np.maximum(h1, 0.0)
    return (h1 @ W2.T + b2).astype(np.float32)


# revision 11
# speedup vs baseline: 1.3765x; 1.1349x over previous
"""Trainium2 Bass kernel for nn_BASE_MAMBA_14018773254552.

Mamba block (d_model=128, d_inner=256, d_state=64, d_conv=4, L=1024, B=4)
+ input proj + classifier head.

Sharding: 8 cores = 4 batches x 2 d_inner-halves (128 channels each).
Each core computes its batch's full front-end (input proj, in_proj, conv,
x_proj) feature-major ([feature, time] tiles), then the selective scan for
its 128-channel half, and the partial out-proj + mean-pool. The host sums
the two channel-half partials per batch and runs the tiny classifier
(BatchNorm couples batches, so it cannot live on one core).

Self-contained: hardcodes all shapes; builds + compiles the Bass program
once per process and runs it on cores 0-7 via run_bass_kernel_spmd.
"""
import numpy as np

try:
    import concourse.bacc as bacc
except ImportError:  # pragma: no cover - path fallback
    import sys
    for _p in ("/opt/trn_rl_repo", "/root/.axon_site/_ro/trn_rl_repo"):
        if _p not in sys.path:
            sys.path.insert(0, _p)
    import concourse.bacc as bacc

import ml_dtypes
import concourse.bass as bass
import concourse.mybir as mybir
import concourse.tile as tile
from concourse.bass_utils import run_bass_kernel_spmd

F32 = mybir.dt.float32
BF16 = mybir.dt.bfloat16
AF = mybir.ActivationFunctionType
OP = mybir.AluOpType

B, L, CIN = 4, 1024, 20
DM, DS, DC = 128, 64, 4
DI = 256
DTR = 8
DH = 128          # channels per core (d_inner half)
EPS = 1e-5

_cache = {}


def _build():
    nc = bacc.Bacc("TRN2", target_bir_lowering=False, debug=False, num_devices=8)

    # ---- I/O ----
    xt_d = nc.dram_tensor("xt", [CIN, L], F32, kind="ExternalInput")
    wpT_d = nc.dram_tensor("wpT", [CIN, DM], F32, kind="ExternalInput")
    bp_d = nc.dram_tensor("bp", [DM, 1], F32, kind="ExternalInput")
    wiT_d = nc.dram_tensor("wiT", [DM, 3 * DH], F32, kind="ExternalInput")
    convw_d = nc.dram_tensor("convw", [DH, 2 * DC], F32, kind="ExternalInput")
    convb_d = nc.dram_tensor("convb", [DH, 2], F32, kind="ExternalInput")
    wxT_d = nc.dram_tensor("wxT", [DH, 2 * 136], F32, kind="ExternalInput")
    wdtT_d = nc.dram_tensor("wdtT", [DTR, DH], F32, kind="ExternalInput")
    bdt_d = nc.dram_tensor("bdt", [DH, 1], F32, kind="ExternalInput")
    alogp_d = nc.dram_tensor("alogp", [DH, DS], F32, kind="ExternalInput")
    dskip_d = nc.dram_tensor("dskip", [DH, 1], F32, kind="ExternalInput")
    woutT_d = nc.dram_tensor("woutT", [DH, DM], F32, kind="ExternalInput")
    selE_d = nc.dram_tensor("selE", [DH, DS * DH], BF16, kind="ExternalInput")
    pooled_d = nc.dram_tensor("pooled", [DM, 1], F32, kind="ExternalOutput")
    dt_scr = nc.dram_tensor("dt_scr", [DH, L], BF16)
    u_scr = nc.dram_tensor("u_scr", [DH, L], BF16)
    bm_scr = nc.dram_tensor("bm_scr", [DS, L], BF16)
    cm_scr = nc.dram_tensor("cm_scr", [DS, L], BF16)

    with tile.TileContext(nc) as tc:
        with (
            tc.tile_pool(name="const", bufs=1) as cp,
            tc.tile_pool(name="work", bufs=1) as wp,
        ):
            # ---- load params ----
            xt = cp.tile([CIN, L], F32)
            wpT = cp.tile([CIN, DM], F32)
            bp = cp.tile([DM, 1], F32)
            wiT = cp.tile([DM, 3 * DH], F32)
            convw = cp.tile([DH, 2 * DC], F32)
            convb = cp.tile([DH, 2], F32)
            wxT = cp.tile([DH, 2 * 136], F32)
            wdtT = cp.tile([DTR, DH], F32)
            bdt = cp.tile([DH, 1], F32)
            alogp = cp.tile([DH, DS], F32)
            dskip = cp.tile([DH, 1], F32)
            woutT = cp.tile([DH, DM], F32)
            selE = cp.tile([DH, DS * DH], BF16)
            for t_, d_ in [(xt, xt_d), (wpT, wpT_d), (bp, bp_d), (wiT, wiT_d),
                           (convw, convw_d), (convb, convb_d), (wxT, wxT_d),
                           (wdtT, wdtT_d), (bdt, bdt_d), (alogp, alogp_d),
                           (dskip, dskip_d), (woutT, woutT_d), (selE, selE_d)]:
                nc.sync.dma_start(t_[:], d_[:])

            HLF = (slice(0, 512), slice(512, 1024))

            # ---- phase 1: front-end ----
            with tc.tile_pool(name="ps1", bufs=4, space="PSUM") as ps1:
                # h = Wp @ x + bp   [128 dm, 1024 t]
                h_ps = ps1.tile([DM, L], F32, tag="ps")
                for sl in HLF:
                    nc.tensor.matmul(h_ps[:, sl], wpT[:, :], xt[:, sl])
                h16 = wp.tile([DM, L], F32)
                nc.scalar.activation(h16[:], h_ps[:], AF.Identity, bias=bp[:])

                # xm_j = W_in[chunk_j] @ h   (j=0 own, j=1 other)
                xmp = []   # padded copies in SBUF
                for j in range(2):
                    xm_ps = ps1.tile([DH, L], F32, tag="ps")
                    for sl in HLF:
                        nc.tensor.matmul(
                            xm_ps[:, sl], wiT[:, j * DH:(j + 1) * DH], h16[:, sl])
                    pad = wp.tile([DH, DC - 1 + L], F32, tag=f"xmp{j}")
                    nc.vector.memset(pad[:, 0:DC - 1], 0.0)
                    nc.scalar.copy(pad[:, DC - 1:DC - 1 + L], xm_ps[:])
                    xmp.append(pad)

                # causal depthwise conv + silu -> xc16_j
                xc16 = []
                for j in range(2):
                    c01 = wp.tile([DH, L], F32, tag=f"c01_{j}")
                    nc.vector.tensor_scalar(
                        out=c01[:], in0=xmp[j][:, 0:L],
                        scalar1=convw[:, 4 * j:4 * j + 1], scalar2=None,
                        op0=OP.mult)
                    nc.vector.scalar_tensor_tensor(
                        out=c01[:], in0=xmp[j][:, 1:1 + L],
                        scalar=convw[:, 4 * j + 1:4 * j + 2],
                        in1=c01[:], op0=OP.mult, op1=OP.add)
                    c23 = wp.tile([DH, L], F32, tag=f"c23_{j}")
                    nc.vector.tensor_scalar(
                        out=c23[:], in0=xmp[j][:, 2:2 + L],
                        scalar1=convw[:, 4 * j + 2:4 * j + 3], scalar2=None,
                        op0=OP.mult)
                    nc.vector.scalar_tensor_tensor(
                        out=c23[:], in0=xmp[j][:, 3:3 + L],
                        scalar=convw[:, 4 * j + 3:4 * j + 4],
                        in1=c23[:], op0=OP.mult, op1=OP.add)
                    cacc = wp.tile([DH, L], F32, tag=f"cacc{j}")
                    nc.vector.tensor_tensor(out=cacc[:], in0=c01[:],
                                            in1=c23[:], op=OP.add)
                    xc = wp.tile([DH, L], F32, tag=f"xc{j}")
                    nc.scalar.activation(xc[:], cacc[:], AF.Silu,
                                         bias=convb[:, j:j + 1])
                    xc16.append(xc)

                # z-gate early (same Silu table as xc): y*silu(z) later
                zsig = wp.tile([DH, L], F32)
                z_ps = ps1.tile([DH, L], F32, tag="ps")
                for sl in HLF:
                    nc.tensor.matmul(z_ps[:, sl], wiT[:, 2 * DH:3 * DH],
                                     h16[:, sl])
                nc.scalar.activation(zsig[:], z_ps[:], AF.Silu)

                # dbc = W_x @ xc  -> dtr [8,L], BmT [64,L], CmT [64,L]
                dtr_ps = ps1.tile([DTR, L], F32, tag="ps")
                bm_ps = ps1.tile([DS, L], F32, tag="ps")
                cm_ps = ps1.tile([DS, L], F32, tag="ps")
                for (m0, msz, out_ps) in ((0, DTR, dtr_ps), (DTR, DS, bm_ps),
                                          (DTR + DS, DS, cm_ps)):
                    for sl in HLF:
                        for j in range(2):
                            nc.tensor.matmul(
                                out_ps[:, sl],
                                wxT[:, 136 * j + m0:136 * j + m0 + msz],
                                xc16[j][:, sl],
                                start=(j == 0), stop=(j == 1))
                dtrT = wp.tile([DTR, L], F32)
                nc.scalar.copy(dtrT[:], dtr_ps[:])
                bmT16 = wp.tile([DS, L], BF16)
                nc.scalar.copy(bmT16[:], bm_ps[:])
                cmT16 = wp.tile([DS, L], BF16)
                nc.scalar.copy(cmT16[:], cm_ps[:])

                # dt = softplus(W_dt @ dtr + b_dt); no Softplus ACT table on
                # TRN2, so compute dt_neg = -dt = ln(sigmoid(-(raw + b_dt)))
                # and pair it with +exp(A_log) in the scan exponentials.
                dt_ps = ps1.tile([DH, L], F32, tag="ps")
                for sl in HLF:
                    nc.tensor.matmul(dt_ps[:, sl], wdtT[:, :], dtrT[:, sl])
                bdtn = wp.tile([DH, 1], F32)
                nc.scalar.mul(bdtn[:], bdt[:], -1.0)
                sg = wp.tile([DH, L], F32)
                nc.scalar.activation(sg[:], dt_ps[:], AF.Sigmoid,
                                     bias=bdtn[:], scale=-1.0)
                DT = wp.tile([DH, L], BF16)  # holds -dt
                nc.scalar.activation(DT[:], sg[:], AF.Ln)

            # U = dt * xc_own (bf16) ;  Apos = exp(A_log) in pair layout
            U = wp.tile([DH, L], BF16)
            nc.vector.scalar_tensor_tensor(
                out=U[:], in0=DT[:], scalar=-1.0, in1=xc16[0][:],
                op0=OP.mult, op1=OP.mult)
            aposp = wp.tile([DH, DS], F32)   # +exp(A_log), pairs with -dt
            nc.scalar.activation(aposp[:], alogp[:], AF.Exp)

            # bounce DT/U to DRAM for pair-replication reads
            nc.sync.dma_start(dt_scr[:], DT[:])
            nc.sync.dma_start(u_scr[:], U[:])
            # Bm2/Cm2: [128, L] bf16, partition q = row q//2 of BmT/CmT
            nc.sync.dma_start(bm_scr[:], bmT16[:])
            nc.sync.dma_start(cm_scr[:], cmT16[:])
            Bm2 = wp.tile([DH, L], BF16)
            Cm2 = wp.tile([DH, L], BF16)
            for scr, dst in ((bm_scr, Bm2), (cm_scr, Cm2)):
                sap = scr[:]
                nc.sync.dma_start(dst[:], bass.AP(
                    tensor=sap.tensor, offset=sap.offset,
                    ap=[sap.ap[0], [0, 2], sap.ap[1]]))

            # ---- phase 2: selective scan, pair layout (q = 2n + j) ----
            # pair p covers channels d0=2p, d1=2p+1; partitions hold (n, j)
            NP = DH // 2          # 64 pairs
            GP_HC = 0             # of every 16 pairs, this many HC on gpsimd
            with (
                tc.tile_pool(name="psl", bufs=1, space="PSUM") as psl,
                tc.tile_pool(name="sl", bufs=5) as slp,
            ):
                Y_ps = psl.tile([DH, L], F32, tag="Y")
                for p in range(NP):
                    dtrep = slp.tile([DH, L], BF16, tag="dtrep")
                    sap = dt_scr[:]
                    nc.sync.dma_start(dtrep[:], bass.AP(
                        tensor=sap.tensor, offset=sap.offset + 2 * p * L,
                        ap=[[0, DS], [L, 2], [1, L]]))
                    urep = slp.tile([DH, L], BF16, tag="urep")
                    sap = u_scr[:]
                    nc.sync.dma_start(urep[:], bass.AP(
                        tensor=sap.tensor, offset=sap.offset + 2 * p * L,
                        ap=[[0, DS], [L, 2], [1, L]]))
                    dAt = slp.tile([DH, L], F32, tag="dA")
                    nc.scalar.activation(dAt[:], dtrep[:], AF.Exp,
                                         scale=aposp[:, p:p + 1])
                    dBxt = slp.tile([DH, L], BF16, tag="dBx")
                    nc.vector.tensor_tensor(out=dBxt[:], in0=urep[:],
                                            in1=Bm2[:], op=OP.mult)
                    Ht = slp.tile([DH, L], BF16, tag="H")
                    nc.vector.tensor_tensor_scan(
                        out=Ht[:], data0=dAt[:], data1=dBxt[:], initial=0.0,
                        op0=OP.mult, op1=OP.add)
                    HCt = slp.tile([DH, L], BF16, tag="HC")
                    eng = nc.gpsimd if (p % 16) < GP_HC else nc.vector
                    eng.tensor_tensor(out=HCt[:], in0=Ht[:], in1=Cm2[:],
                                      op=OP.mult)
                    selp = selE[:, DH * p:DH * (p + 1)]
                    for sl in HLF:
                        nc.tensor.matmul(Y_ps[:, sl], selp, HCt[:, sl],
                                         start=(p == 0), stop=(p == NP - 1))

            # ---- tail: gate, out-proj, pool (pipelined by t-half) ----
                y2 = wp.tile([DH, L], F32)
                y3 = wp.tile([DH, L], F32)
                trash = wp.tile([DM, L], F32)
                pooled_h = wp.tile([DM, 2], F32)
                pooled = wp.tile([DM, 1], F32)
                with tc.tile_pool(name="ps2", bufs=2, space="PSUM") as ps2:
                    out_ps = ps2.tile([DM, L], F32, tag="o")
                    for hi, sl in enumerate(HLF):
                        nc.vector.scalar_tensor_tensor(
                            out=y2[:, sl], in0=xc16[0][:, sl], scalar=dskip[:],
                            in1=Y_ps[:, sl], op0=OP.mult, op1=OP.add)
                        nc.vector.tensor_tensor(out=y3[:, sl], in0=y2[:, sl],
                                                in1=zsig[:, sl], op=OP.mult)
                        nc.tensor.matmul(out_ps[:, sl], woutT[:, :], y3[:, sl])
                        nc.scalar.activation(
                            trash[:, sl], out_ps[:, sl], AF.Identity,
                            scale=1.0 / L, accum_out=pooled_h[:, hi:hi + 1])
                    nc.vector.tensor_tensor(
                        out=pooled[:], in0=pooled_h[:, 0:1],
                        in1=pooled_h[:, 1:2], op=OP.add)
                    nc.sync.dma_start(pooled_d[:], pooled[:])

    nc.compile()
    return nc


def _core_inputs(inputs, b, half):
    f32 = np.float32
    bf16 = ml_dtypes.bfloat16
    x = np.asarray(inputs["x"], f32)
    Wp = np.asarray(inputs["Wp"], f32)
    bp = np.asarray(inputs["bp"], f32)
    W_in = np.asarray(inputs["W_in"], f32)
    conv_w = np.asarray(inputs["conv_w"], f32)
    conv_b = np.asarray(inputs["conv_b"], f32)
    W_x = np.asarray(inputs["W_x"], f32)
    W_dt = np.asarray(inputs["W_dt"], f32)
    b_dt = np.asarray(inputs["b_dt"], f32)
    A_log = np.asarray(inputs["A_log"], f32)
    Dskip = np.asarray(inputs["Dskip"], f32)
    W_out = np.asarray(inputs["W_out"], f32)

    own = slice(half * DH, half * DH + DH)
    other = slice(DH, 2 * DH) if half == 0 else slice(0, DH)
    return {
        "xt": np.ascontiguousarray(x[b]),
        "wpT": np.ascontiguousarray(Wp.T),
        "bp": np.ascontiguousarray(bp[:, None]),
        "wiT": np.concatenate(
            [W_in[0:DI][own].T, W_in[0:DI][other].T,
             W_in[DI:2 * DI][own].T], axis=1),
        "convw": np.concatenate([conv_w[own], conv_w[other]], axis=1),
        "convb": np.stack([conv_b[own], conv_b[other]], axis=1),
        "wxT": np.concatenate([W_x.T[own], W_x.T[other]], axis=1),
        "wdtT": np.ascontiguousarray(W_dt[own].T),
        "bdt": np.ascontiguousarray(b_dt[own][:, None]),
        "alogp": _alog_pairs(A_log[own]),
        "dskip": np.ascontiguousarray(Dskip[own][:, None]),
        "woutT": np.ascontiguousarray(W_out[:, own].T),
        "selE": _selE(),
    }


def _alog_pairs(alog_own):
    # alogp[q, p] = A_log[own][2p + q%2, q//2]
    out = np.empty((DH, DS), np.float32)
    q = np.arange(DH)
    for p in range(DS):
        out[:, p] = alog_own[2 * p + (q % 2), q // 2]
    return out


_selE_cache = {}


def _selE():
    if "v" not in _selE_cache:
        sel = np.zeros((DH, DS * DH), np.float32)
        q = np.arange(DH)
        for p in range(DS):
            sel[q, DH * p + 2 * p + (q % 2)] = 1.0
        _selE_cache["v"] = sel.astype(ml_dtypes.bfloat16)
    return _selE_cache["v"]


def kernel(**inputs) -> np.ndarray:
    if "nc" not in _cache:
        _cache["nc"] = _build()
    nc = _cache["nc"]

    in_maps = [_core_inputs(inputs, c // 2, c % 2) for c in range(8)]
    res = run_bass_kernel_spmd(nc, in_maps, core_ids=list(range(8)))

    pooled = np.zeros((B, DM), np.float32)
    for c in range(8):
        pooled[c // 2] += res.results[c]["pooled"][:, 0]

    # classifier head (host: BatchNorm couples all batches; ~300 flops)
    f32 = np.float32
    W1 = np.asarray(inputs["W1"], f32)
    b1 = np.asarray(inputs["b1"], f32)
    gamma = np.asarray(inputs["gamma"], f32)
    beta = np.asarray(inputs["beta"], f32)
    W2 = np.asarray(inputs["W2"], f32)
    b2 = np.asarray(inputs["b2"], f32)
    h1 = pooled @ W1.T + b1
    mu = h1.mean(axis=0)
    var = h1.var(axis=0)
    h1 = (h1 - mu) / np.sqrt(var + EPS) * gamma + beta
    h1 = np.maximum(h1, 0.0)
    return (h1 @ W2.T + b2).astype(np.float32)
